# revision 26
# baseline (speedup 1.0000x reference)
"""Bahdanau attention cell (location-sensitive) on 8 TRN2 NeuronCores.

Sharding: data-parallel over the batch dim (64 -> 8 batches/core); all
params (conv kernel, location dense, score v/b) are tiny and replicated.

Per-core device program (Bass/Tile), [t-on-partitions, a-on-free] layout:
  1. conv(prev_weights) as a block-diagonal matmul over an im2col matrix
     (2 batches per 66-row group; rows 32/65 become constant-1 rows via the
     evacuation bias so the location matmul can fold in query+score_b).
  2. main pass per (batch, 512-t group):
       PSUM[t, a]  = f-chunk.T @ [loc_w ; q+b]      (ploc + query, fp32r)
                   + ident @ w[t, a]                 (w accumulate, fp32r)
       tanh on ACT -> bf16, then DVE tensor_tensor_reduce with score_v
       -> energy column e[t-chunk].
  3. energy columns -> PE transpose -> SBUF->SBUF DMA rearrange to [b, T],
     masked softmax over t on DVE/ACT/GpSimd, cumulative add, DMA out.
"""

import sys

sys.path.insert(0, "/opt/trn_rl_repo")

import numpy as np

import concourse.bacc as bacc
import concourse.bass as bass
import concourse.tile as tile
from concourse import mybir
from concourse.bass_utils import run_bass_kernel_spmd

B, T, A, F, KW = 64, 2000, 256, 32, 31
NCORES = 8
BL = B // NCORES  # 8 batches per core
PAD = (KW - 1) // 2  # 15
TP = T + 2 * PAD  # 2030
TC = 2048  # t padded to 16 chunks of 128
TP2 = TC + 2 * PAD  # padded prev: conv runs over all TC columns
NCH = TC // 128  # 16 chunks per batch
NG = 4  # groups of 4 chunks (512 t) per batch
F32 = mybir.dt.float32
F32R = mybir.dt.float32r
F16 = mybir.dt.float16
BF16 = mybir.dt.bfloat16

TR = [(0, 1024), (1024, 1024)]

PROBE = "full"  # bisect knob: full|nottr|norearr|syncdma|th32


def _halves(tsz):
    out = [(0, min(512, tsz))]
    if tsz > 512:
        out.append((512, tsz - 512))
    return out


def _r(ap):
    return ap.bitcast(F32R)


def build_program():
    nc = bacc.Bacc("TRN2", target_bir_lowering=False)

    wmem = nc.dram_tensor("wmem", [BL, TC, A], F32, kind="ExternalInput")
    prevp = nc.dram_tensor("prevp", [BL, TP2], F32, kind="ExternalInput")
    locq = nc.dram_tensor("locq", [98, BL * A], F32, kind="ExternalInput")
    bd = nc.dram_tensor("bd", [2 * KW, 98], F32, kind="ExternalInput")
    cb66 = nc.dram_tensor("cb66", [98, 1], F32, kind="ExternalInput")
    vrep = nc.dram_tensor("vrep", [128, A], F32, kind="ExternalInput")
    ident = nc.dram_tensor("ident", [128, 128], F32, kind="ExternalInput")
    maskd = nc.dram_tensor("mask", [BL, T], F32, kind="ExternalInput")
    out_w = nc.dram_tensor("out_w", [BL, T], F32, kind="ExternalOutput")
    out_nw = nc.dram_tensor("out_nw", [BL, T], F32, kind="ExternalOutput")

    with tile.TileContext(nc) as tc:
        with (
            tc.tile_pool(name="singles", bufs=1) as singles,
            tc.tile_pool(name="impool", bufs=4) as impool,
            tc.tile_pool(name="wpool", bufs=4) as wpool,
            tc.tile_pool(name="thpool", bufs=4) as thpool,
            tc.tile_pool(name="scrpool", bufs=2) as scrpool,
            tc.tile_pool(name="spool", bufs=1) as spool,
            tc.tile_pool(name="pz", bufs=4, space="PSUM") as pzpool,
        ):
            # ---- w prefetch: all batches, first in the gpsimd DMA queue ----
            w_tiles = []
            for b in range(BL):
                w_sb = wpool.tile([128, NCH * A], BF16, tag="w")
                w_tiles.append(w_sb)
                base = wmem[b, :, :]
                wsrc = bass.AP(
                    tensor=base.tensor,
                    offset=base.offset,
                    ap=[[A, 128], [128 * A, NCH], [1, A]],
                )
                nc.gpsimd.dma_start(out=w_sb[:], in_=wsrc)

            # ---- constants (scalar/sync HW DGE queues + DVE converts) ----
            ident32 = singles.tile([128, 128], F32, tag="ident32")
            nc.scalar.dma_start(out=ident32[:], in_=ident[:])
            ident16 = singles.tile([128, 128], BF16, tag="ident16")
            nc.vector.tensor_copy(out=ident16[:], in_=ident32[:])
            bd_f = singles.tile([2 * KW, 98], F32, tag="bd_f")
            nc.scalar.dma_start(out=bd_f[:], in_=bd[:])
            bd_sb = singles.tile([2 * KW, 98], BF16, tag="bd")
            nc.vector.tensor_copy(out=bd_sb[:], in_=bd_f[:])
            cb_sb = singles.tile([98, 1], F32, tag="cb")
            nc.scalar.dma_start(out=cb_sb[:], in_=cb66[:])
            locq_f = singles.tile([98, BL * A], F32, tag="locq_f")
            nc.scalar.dma_start(out=locq_f[:], in_=locq[:])
            locq_sb = singles.tile([98, BL * A], BF16, tag="locq")
            nc.vector.tensor_copy(out=locq_sb[:], in_=locq_f[:])
            vrep_sb = singles.tile([128, A], F32, tag="vrep")
            nc.scalar.dma_start(out=vrep_sb[:], in_=vrep[:])
            vrep4 = singles.tile([128, 4 * A], F16, tag="vrep4")
            for _k in range(4):
                nc.vector.tensor_copy(
                    out=vrep4[:, _k * A : (_k + 1) * A], in_=vrep_sb[:]
                )
            mask_sb = singles.tile([BL, T], F32, tag="mask")
            nc.sync.dma_start(out=mask_sb[:], in_=maskd[:])
            prev_sb = singles.tile([BL, T], F32, tag="prev")
            nc.sync.dma_start(out=prev_sb[:], in_=prevp[:, PAD : PAD + T])
            # early engine touches: let DVE/GpSimd observe these DMA sems up
            # front so late consumers carry few fresh sync waits each.
            warm = singles.tile([BL, 2], F32, tag="warm")
            nc.vector.tensor_copy(out=warm[:, 0:1], in_=mask_sb[:, 0:1])
            nc.vector.tensor_copy(out=warm[:, 1:2], in_=prev_sb[:, 0:1])

            # ---- conv phase: f[g] [98, TC]; rows 0-31 f(even batch),
            #      rows 32/33 ones, rows 64-95 f(odd), rows 96/97 ones ----
            f_sb = []
            for g in range(NG):
                fg = singles.tile([98, TC], BF16, tag=f"f{g}")
                f_sb.append(fg)
                im_f = impool.tile([2 * KW, TC], F32, tag="im_f")
                base = prevp[2 * g : 2 * g + 2, :]
                src = bass.AP(
                    tensor=base.tensor,
                    offset=base.offset,
                    ap=[[TP2, 2], [1, KW], [1, TC]],
                )
                nc.scalar.dma_start(out=im_f[:], in_=src)
                im = impool.tile([2 * KW, TC], BF16, tag="im")
                nc.vector.tensor_copy(out=im[:], in_=im_f[:])
                for t0, tsz in TR:
                    pc = pzpool.tile([128, 1024], F32, tag="z")
                    for u0, un in _halves(tsz):
                        nc.tensor.matmul(
                            pc[0:98, u0 : u0 + un],
                            bd_sb[:],
                            im[:, t0 + u0 : t0 + u0 + un],
                            start=True,
                            stop=True,
                        )
                    # evacuate with conv bias; row 32 has zero psum and
                    # bias 1.0 -> becomes the constant-1 row for the q fold
                    nc.scalar.activation(
                        out=fg[:, t0 : t0 + tsz],
                        in_=pc[0:98, 0:tsz],
                        func=mybir.ActivationFunctionType.Identity,
                        bias=cb_sb[:, 0:1],
                        scale=1.0,
                    )

            # ---- main pass ----
            e_cols = spool.tile([128, 128], F32, tag="e_cols")
            for b in range(BL):
                w_sb = w_tiles[b]
                r0 = 64 * (b % 2)
                fg = f_sb[b // 2]
                for g in range(NG):
                    pzt = pzpool.tile([128, 1024], F32, tag="z")
                    for u0 in (0, 512):
                        nc.tensor.matmul(
                            pzt[:, u0 : u0 + 512],
                            ident16[:],
                            w_sb[:, g * 1024 + u0 : g * 1024 + u0 + 512],
                            start=True,
                            stop=False,
                        )
                        for j in (u0 // A, u0 // A + 1):
                            ch = g * 4 + j
                            nc.tensor.matmul(
                                pzt[:, j * A : (j + 1) * A],
                                fg[r0 : r0 + 34, ch * 128 : (ch + 1) * 128],
                                locq_sb[r0 : r0 + 34, b * A : (b + 1) * A],
                                start=False,
                                stop=(j % 2 == 1),
                            )
                    th = thpool.tile([128, 1024], F16, tag="th")
                    nc.scalar.activation(
                        out=th[:],
                        in_=pzt[:],
                        func=mybir.ActivationFunctionType.Tanh,
                    )
                    col0 = b * NCH + g * 4
                    y = scrpool.tile([128, 1024], F16, tag="y")
                    nc.vector.tensor_mul(y[:], th[:], vrep4[:])
                    nc.vector.tensor_reduce(
                        out=e_cols[:, col0 : col0 + 4],
                        in_=y[:].rearrange("p (j a) -> p j a", j=4),
                        axis=mybir.AxisListType.X,
                        op=mybir.AluOpType.add,
                    )

            # ---- energy columns -> [BL, T] rows ----
            pe_t = pzpool.tile([128, 1024], F32, tag="z")
            nc.tensor.matmul(
                pe_t[:, 0:128], e_cols[:], ident32[:], is_transpose=True,
                start=True, stop=True,
            )
            e_rows = spool.tile([128, 128], F32, tag="e_rows")
            nc.vector.tensor_copy(out=e_rows[:], in_=pe_t[:, 0:128])
            energy_sb = spool.tile([BL, TC], F32, tag="energy")
            if PROBE == "norearr":
                nc.vector.tensor_copy(out=energy_sb[0:8, 0:128], in_=e_rows[0:8, :])
            else:
                nc.sync.dma_start(out=energy_sb[:], in_=e_rows[:])
            energy = energy_sb[:, 0:T]

            # ---- masked softmax + cumulative add ----
            scratch = spool.tile([BL, T], F32, tag="scratch")
            nc.vector.tensor_mul(scratch[:], energy, mask_sb[:])
            mx = spool.tile([BL, 1], F32, tag="mx")
            nc.vector.tensor_reduce(
                out=mx[:], in_=scratch[:],
                axis=mybir.AxisListType.X, op=mybir.AluOpType.max,
            )
            negm = spool.tile([BL, 1], F32, tag="negm")
            nc.vector.tensor_scalar_mul(negm[:], mx[:], -1.0)
            e_sb = spool.tile([BL, T], F32, tag="e")
            nc.scalar.activation(
                out=e_sb[:],
                in_=energy,
                func=mybir.ActivationFunctionType.Exp,
                bias=negm[:, 0:1],
                scale=1.0,
            )
            num_sb = spool.tile([BL, T], F32, tag="num")
            ssum = spool.tile([BL, 1], F32, tag="ssum")
            nc.vector.tensor_mul(num_sb[:], e_sb[:], mask_sb[:])
            nc.vector.tensor_reduce(
                out=ssum[:], in_=num_sb[:],
                axis=mybir.AxisListType.X, op=mybir.AluOpType.add,
            )
            rinv = spool.tile([BL, 1], F32, tag="rinv")
            nc.vector.reciprocal(rinv[:], ssum[:])
            ow_sb = spool.tile([BL, T], F32, tag="ow")
            nc.vector.tensor_scalar_mul(ow_sb[:], num_sb[:], rinv[:, 0:1])
            nw_sb = spool.tile([BL, T], F32, tag="nw")
            nc.vector.tensor_add(nw_sb[:], ow_sb[:], prev_sb[:])
            nc.sync.dma_start(out=out_w[:], in_=ow_sb[:])
            nc.sync.dma_start(out=out_nw[:], in_=nw_sb[:])

    nc.finalize()
    return nc


def make_in_maps(query, prev_weights, w_memory, memory_lengths, conv_w, conv_b,
                 loc_w, score_v, score_b):
    """Host-side prep (small params only) + batch sharding."""
    query = np.asarray(query, np.float32)
    prev_weights = np.asarray(prev_weights, np.float32)
    w_memory = np.asarray(w_memory, np.float32)
    memory_lengths = np.asarray(memory_lengths)
    conv_w = np.asarray(conv_w, np.float32)
    conv_b = np.asarray(conv_b, np.float32)
    loc_w = np.asarray(loc_w, np.float32)
    score_v = np.asarray(score_v, np.float32)
    score_b = np.asarray(score_b, np.float32)

    # block-diagonal conv kernel: even batch -> cols 0..31, odd -> 64..95;
    # cols 32/33/96/97 stay zero so the evac bias of 1.0 builds the
    # constant-1 rows used to fold in q (split into two bf16 rows).
    bd = np.zeros((2 * KW, 98), np.float32)
    bd[0:KW, 0:F] = conv_w[:, 0, :]
    bd[KW : 2 * KW, 64 : 64 + F] = conv_w[:, 0, :]
    cb66 = np.zeros((98, 1), np.float32)
    cb66[0:F, 0] = conv_b
    cb66[64 : 64 + F, 0] = conv_b
    cb66[32, 0] = 1.0
    cb66[33, 0] = 1.0
    cb66[96, 0] = 1.0
    cb66[97, 0] = 1.0
    vrep = np.tile(score_v[None, :], (128, 1)).astype(np.float32)
    ident = np.eye(128, dtype=np.float32)
    qb = query + score_b[None, :]  # [B, A]
    prevp_full = np.pad(prev_weights, ((0, 0), (PAD, TC - T + PAD)))
    mask_full = (
        np.arange(T)[None, :] < memory_lengths[:, None]
    ).astype(np.float32)

    in_maps = []
    for i in range(NCORES):
        s = slice(i * BL, (i + 1) * BL)
        wshard = np.zeros((BL, TC, A), np.float32)
        wshard[:, :T, :] = w_memory[s]
        # moving operand mirrors the f-row structure (same base partition):
        # even batch reads rows 0..33, odd batch rows 64..97; q+score_b is
        # split into two bf16-exact rows (hi + residual) for full precision.
        def _bf16(x):
            u = x.astype(np.float32).view(np.uint32)
            u = (u + 0x8000 + ((u >> 16) & 1)) & 0xFFFF0000
            return u.view(np.float32)

        locq_c = np.zeros((98, BL * A), np.float32)
        for b in range(BL):
            cs = slice(b * A, (b + 1) * A)
            r0 = 64 * (b % 2)
            q_hi = _bf16(qb[i * BL + b])
            q_lo = _bf16(qb[i * BL + b] - q_hi)
            locq_c[r0 : r0 + F, cs] = loc_w
            locq_c[r0 + 32, cs] = q_hi
            locq_c[r0 + 33, cs] = q_lo
        in_maps.append(
            {
                "wmem": wshard,
                "prevp": np.ascontiguousarray(prevp_full[s]),
                "locq": locq_c,
                "bd": bd,
                "cb66": cb66,
                "vrep": vrep,
                "ident": ident,
                "mask": np.ascontiguousarray(mask_full[s]),
            }
        )
    return in_maps


_NC_CACHE = {}


def _get_nc():
    if "nc" not in _NC_CACHE:
        _NC_CACHE["nc"] = build_program()
    return _NC_CACHE["nc"]


def run(inputs, trace=False, tmpdir=None):
    """Run on 8 NeuronCores; returns ((output, new_weights), BassKernelResults)."""
    nc = _get_nc()
    in_maps = make_in_maps(**inputs)
    res = run_bass_kernel_spmd(
        nc, in_maps, core_ids=list(range(NCORES)), trace=trace, tmpdir=tmpdir
    )
    output = np.concatenate([res.results[i]["out_w"] for i in range(NCORES)], axis=0)
    new_w = np.concatenate([res.results[i]["out_nw"] for i in range(NCORES)], axis=0)
    return (output.astype(np.float32), new_w.astype(np.float32)), res


def kernel(**inputs):
    (output, new_w), _ = run(inputs, trace=False)
    return output, new_w
